# revision 2
# baseline (speedup 1.0000x reference)
"""ModAFNO2D layer as a Bass/Tile kernel for 8 Trainium2 NeuronCores.

Sharding: 8 cores = (batch b in 0..3) x (block-half in 0..1). Each core owns one
batch sample and 4 of the 8 FNO blocks (= 384 of 768 channels). The FFT axes are
per-channel and channel blocks never mix, so cores are fully independent — no
collectives; host slices inputs and concatenates outputs.

Per-core pipeline (all heavy math on the PE as matmuls; DFT done as matrix
multiply with precomputed 128-point DFT matrices):
  A : Z^T = X_c^T @ [Fr|Fi]                 (FFT along H; X_c stationary)
  B : Y[c,(Yr|Yi)] at fixed h'              (rFFT along W; Z^T slices stationary)
  mix: block-diagonal 2-layer complex MLP with adaLN modulation (weights
       stationary, channels on partitions), relu + softshrink fused into evicts
  T : PE transposes [c,wf]->[wf,c] to pivot back to spatial-major
  E': [Pr|Pi] = Z @ [Sr|Si]                 (inverse rFFT along W)
  D': out = FHr@Pr - FHi@Pi + x             (inverse FFT along H + residual)
Intermediate spectra are bf16 (the FFT branch contributes ~4% of output scale;
residual path stays fp32), PSUM accumulation fp32.
"""

import numpy as np
import ml_dtypes

BF16 = ml_dtypes.bfloat16

DIM = 768
NB = 8
BS = 96
LAM = 0.01
B_FULL = 4
H = 128
W = 128
WF = W // 2 + 1  # 65
NBL = 4          # blocks per core
C = NBL * BS     # 384 channels per core
N_CORES = 8
HC = 4           # h' rows per fused B/mix/T chunk


def _host_consts():
    jh = np.arange(H)
    F = np.exp(-2j * np.pi * np.outer(jh, jh) / H)
    R = np.exp(-2j * np.pi * np.outer(np.arange(WF), np.arange(W)) / W) / 128.0
    RrT, RiT = R.real.T, R.imag.T                      # [w, wf]
    FH = np.conj(F)
    cw = np.ones(WF)
    cw[1:-1] = 2.0
    S = (cw[:, None] * np.exp(2j * np.pi * np.outer(np.arange(WF), np.arange(W)) / W)) / 128.0
    consts = {
        "cF": np.concatenate([F.real, F.imag], 1).astype(np.float32),      # [128, 256]
        "cB1": np.concatenate([RrT, RiT], 1).astype(BF16),                 # [128, 130]
        "cB2": np.concatenate([-RiT, RrT], 1).astype(BF16),                # [128, 130]
        "cE1": np.concatenate([S.real, S.imag], 1).astype(BF16),           # [65, 256]
        "cE2": np.concatenate([-S.imag, S.real], 1).astype(BF16),          # [65, 256]
        "cDr": FH.real.astype(BF16),                                       # [128, 128]
        "cDi": (-FH.imag).astype(BF16),                                    # [128, 128]
        "cI": np.eye(128, dtype=np.float32),                               # [128, 128]
    }
    return consts


def _build_program():
    import os as _os
    _stages = set(_os.environ.get("K_STAGES", "MAB12TXED").upper())
    _reps = int(_os.environ.get("K_REPS", "1"))
    from contextlib import ExitStack

    import concourse.bass as bass  # noqa: F401
    import concourse.mybir as mybir
    import concourse.tile as tile
    from concourse import bacc

    f32 = mybir.dt.float32
    bf = mybir.dt.bfloat16
    AF = mybir.ActivationFunctionType

    nc = bacc.Bacc("TRN2", target_bir_lowering=False, debug=False)

    xs = nc.dram_tensor("xs", [C, H, W], f32, kind="ExternalInput")
    tb = nc.dram_tensor("tb", [DIM], f32, kind="ExternalInput")
    w1s = nc.dram_tensor("w1s", [2, NBL, BS, BS], f32, kind="ExternalInput")
    b1s = nc.dram_tensor("b1s", [2, NBL, BS], f32, kind="ExternalInput")
    w2s = nc.dram_tensor("w2s", [2, NBL, BS, BS], f32, kind="ExternalInput")
    b2s = nc.dram_tensor("b2s", [2, NBL, BS], f32, kind="ExternalInput")
    mwT = nc.dram_tensor("mwT", [DIM, 2 * NBL * BS], f32, kind="ExternalInput")
    mbs = nc.dram_tensor("mbs", [2 * NBL * BS], f32, kind="ExternalInput")
    cF = nc.dram_tensor("cF", [H, 2 * H], f32, kind="ExternalInput")
    cB1 = nc.dram_tensor("cB1", [W, 2 * WF], bf, kind="ExternalInput")
    cB2 = nc.dram_tensor("cB2", [W, 2 * WF], bf, kind="ExternalInput")
    cE1 = nc.dram_tensor("cE1", [WF, 2 * W], bf, kind="ExternalInput")
    cE2 = nc.dram_tensor("cE2", [WF, 2 * W], bf, kind="ExternalInput")
    cDr = nc.dram_tensor("cDr", [H, H], bf, kind="ExternalInput")
    cDi = nc.dram_tensor("cDi", [H, H], bf, kind="ExternalInput")
    cI = nc.dram_tensor("cI", [128, 128], f32, kind="ExternalInput")
    outs = nc.dram_tensor("outs", [C, H, W], f32, kind="ExternalOutput")

    with ExitStack() as ctx:
        tc = ctx.enter_context(tile.TileContext(nc))
        consts = ctx.enter_context(tc.tile_pool(name="consts", bufs=1))
        blockp = ctx.enter_context(tc.tile_pool(name="blockp", bufs=1))
        xstage = ctx.enter_context(tc.tile_pool(name="xstage", bufs=1))
        mixp = ctx.enter_context(tc.tile_pool(name="mixp", bufs=2))
        outp = ctx.enter_context(tc.tile_pool(name="outp", bufs=3))
        psum = ctx.enter_context(tc.tile_pool(name="psum", bufs=2, space="PSUM"))

        # ---- constants into SBUF ----
        cF_sb = consts.tile([H, 2 * H], f32)
        nc.sync.dma_start(cF_sb, cF[:])
        cB1_sb = consts.tile([W, 2 * WF], bf)
        nc.sync.dma_start(cB1_sb, cB1[:])
        cB2_sb = consts.tile([W, 2 * WF], bf)
        nc.sync.dma_start(cB2_sb, cB2[:])
        cE1_sb = consts.tile([WF, 2 * W], bf)
        nc.sync.dma_start(cE1_sb, cE1[:])
        cE2_sb = consts.tile([WF, 2 * W], bf)
        nc.sync.dma_start(cE2_sb, cE2[:])
        cDr_sb = consts.tile([H, H], bf)
        nc.sync.dma_start(cDr_sb, cDr[:])
        cDi_sb = consts.tile([H, H], bf)
        nc.sync.dma_start(cDi_sb, cDi[:])
        cI_sb = consts.tile([128, 128], f32)
        nc.sync.dma_start(cI_sb, cI[:])

        # ---- block weights (all 4 blocks) ----
        w1r_sb = consts.tile([BS, NBL, BS], f32)
        w1i_sb = consts.tile([BS, NBL, BS], f32)
        nw1i_sb = consts.tile([BS, NBL, BS], f32)
        w2r_sb = consts.tile([BS, NBL, BS], f32)
        w2i_sb = consts.tile([BS, NBL, BS], f32)
        nw2i_sb = consts.tile([BS, NBL, BS], f32)
        nc.sync.dma_start(w1r_sb, w1s[0].rearrange("n d k -> d n k"))
        nc.sync.dma_start(w1i_sb, w1s[1].rearrange("n d k -> d n k"))
        nc.sync.dma_start(w2r_sb, w2s[0].rearrange("n d k -> d n k"))
        nc.sync.dma_start(w2i_sb, w2s[1].rearrange("n d k -> d n k"))
        nc.scalar.mul(nw1i_sb, w1i_sb, -1.0)
        nc.scalar.mul(nw2i_sb, w2i_sb, -1.0)

        # ---- modulation: mod = silu(t) @ mod_w.T + mod_b ----
        modpool_cm = tc.tile_pool(name="modp", bufs=1)
        modpool = modpool_cm.__enter__()
        t_sb = modpool.tile([128, 6], f32)
        nc.sync.dma_start(t_sb, tb[:].rearrange("(j p) -> p j", p=128))
        s_sb = modpool.tile([128, 6], f32)
        nc.scalar.activation(s_sb, t_sb, AF.Silu)
        mwT_sb = modpool.tile([128, 6, 2 * NBL * BS], f32)
        nc.sync.dma_start(mwT_sb, mwT[:].rearrange("(uc p) j -> p uc j", p=128))
        mb_sb = modpool.tile([1, 2 * NBL * BS], f32)
        nc.sync.dma_start(mb_sb, mbs[None, :])
        mod_sb = modpool.tile([1, 2 * NBL * BS], f32)
        nc.vector.memset(mod_sb, 0.0)
        for half in range(2 if "M" in _stages else 0):
            pm = psum.tile([1, 384], f32, tag="ps_m")
            for uc in range(6):
                nc.tensor.matmul(
                    pm,
                    lhsT=s_sb[:, uc : uc + 1],
                    rhs=mwT_sb[:, uc, half * 384 : (half + 1) * 384],
                    start=(uc == 0),
                    stop=(uc == 5),
                )
            nc.vector.tensor_add(
                mod_sb[:, half * 384 : (half + 1) * 384],
                pm,
                mb_sb[:, half * 384 : (half + 1) * 384],
            )

        # per-block modulation vectors: sh' = shift+1, addv = b1*sh' + scale
        lamn = consts.tile([128, 1], f32)
        nc.vector.memset(lamn, -LAM)
        shp1 = consts.tile([BS, NBL], f32)
        scv = consts.tile([BS, NBL], f32)
        addr_v = consts.tile([BS, NBL], f32)
        addi_v = consts.tile([BS, NBL], f32)
        b1r_v = consts.tile([BS, NBL], f32)
        b1i_v = consts.tile([BS, NBL], f32)
        b2r_v = consts.tile([BS, NBL], f32)
        b2iml_v = consts.tile([BS, NBL], f32)   # b2i - lam
        nb2iml_v = consts.tile([BS, NBL], f32)  # -b2i - lam
        nc.sync.dma_start(b1r_v, b1s[0].rearrange("n d -> d n"))
        nc.sync.dma_start(b1i_v, b1s[1].rearrange("n d -> d n"))
        nc.sync.dma_start(b2r_v, b2s[0].rearrange("n d -> d n"))
        b2i_tmp = consts.tile([BS, NBL], f32)
        nc.sync.dma_start(b2i_tmp, b2s[1].rearrange("n d -> d n"))
        nc.scalar.add(b2iml_v, b2i_tmp, lamn[0:BS])
        tmp_nb = consts.tile([BS, NBL], f32)
        nc.scalar.mul(tmp_nb, b2i_tmp, -1.0)
        nc.scalar.add(nb2iml_v, tmp_nb, lamn[0:BS])
        for n in range(NBL):
            nc.sync.dma_start(
                shp1[:, n : n + 1], mod_sb[0:1, n * 192 : n * 192 + 96]
            )
            nc.sync.dma_start(
                scv[:, n : n + 1], mod_sb[0:1, n * 192 + 96 : n * 192 + 192]
            )
        nc.scalar.add(shp1, shp1, 1.0)
        # addv = b1 * shp1 + scale
        nc.vector.tensor_mul(addr_v, b1r_v, shp1)
        nc.vector.tensor_add(addr_v, addr_v, scv)
        nc.vector.tensor_mul(addi_v, b1i_v, shp1)
        nc.vector.tensor_add(addi_v, addi_v, scv)
        modpool_cm.__exit__(None, None, None)

        # ---- main per-block pipeline ----
        rep_cm = tc.For_i(0, _reps, 1) if _reps > 1 else None
        if rep_cm is not None:
            rep_cm.__enter__()
        for n in range(NBL):
            c0 = n * BS

            # resident X for this block: [h, (c, w)] fp32 (stage-A stationary + residual)
            X_blk = blockp.tile([H, BS, W], f32, tag="xblk")
            for c in range(BS):
                nc.sync.dma_start(X_blk[:, c, :], xs[c0 + c])

            # ---- stage A: Z^T = X_c^T @ [Fr|Fi]  -> Zbuf [w, (c, h'Zr|h'Zi)] bf16 ----
            Zbuf = blockp.tile([W, BS, 2 * H], bf, tag="zpbuf")
            for cp in range(BS // 2 if "A" in _stages else 0):
                c = 2 * cp
                pA = psum.tile([128, 2, 2 * H], f32, tag="ps_a")
                nc.tensor.matmul(pA[:, 0, :], lhsT=X_blk[:, c, :], rhs=cF_sb, start=True, stop=True)
                nc.tensor.matmul(pA[:, 1, :], lhsT=X_blk[:, c + 1, :], rhs=cF_sb, start=True, stop=True)
                if cp % 2 == 0:
                    nc.vector.tensor_copy(Zbuf[:, c : c + 2, :], pA)
                else:
                    nc.scalar.copy(Zbuf[:, c : c + 2, :], pA)

            # ---- fused B -> mix -> T, per chunk of HC h' rows ----
            # W planes laid out [wf, h', c] (h' outer) so T evicts merge 4 h' per op
            Wr_pl = blockp.tile([WF, H, BS], bf, tag="wrpl")
            Wi_pl = blockp.tile([WF, H, BS], bf, tag="wipl")
            for ch_i in range(H // HC if "B" in _stages else 0):
                h0 = ch_i * HC
                arch = mixp.tile([BS, HC, 2 * WF], f32, tag="arch")
                for j2 in range(HC // 2):
                    pB = psum.tile([BS, 2, 2 * WF], f32, tag="ps_b")
                    for j in range(2):
                        hj = h0 + j2 * 2 + j
                        nc.tensor.matmul(
                            pB[:, j, :], lhsT=Zbuf[:, :, hj], rhs=cB1_sb,
                            start=True, stop=False,
                        )
                        nc.tensor.matmul(
                            pB[:, j, :], lhsT=Zbuf[:, :, H + hj], rhs=cB2_sb,
                            start=False, stop=True,
                        )
                    if j2 % 2 == 0:
                        nc.vector.tensor_copy(arch[:, j2 * 2 : j2 * 2 + 2, :], pB)
                    else:
                        nc.scalar.copy(arch[:, j2 * 2 : j2 * 2 + 2, :], pB)
                Ar = arch[:, :, 0:WF]
                Ai = arch[:, :, WF : 2 * WF]
                if "1" not in _stages:
                    continue
                # layer 1
                p1r = psum.tile([BS, HC, WF], f32, tag="ps_m")
                nc.tensor.matmul(p1r, lhsT=w1r_sb[:, n, :], rhs=Ar, start=True, stop=False)
                nc.tensor.matmul(p1r, lhsT=nw1i_sb[:, n, :], rhs=Ai, start=False, stop=True)
                p1i = psum.tile([BS, HC, WF], f32, tag="ps_m")
                nc.tensor.matmul(p1i, lhsT=w1i_sb[:, n, :], rhs=Ar, start=True, stop=False)
                nc.tensor.matmul(p1i, lhsT=w1r_sb[:, n, :], rhs=Ai, start=False, stop=True)
                r1 = mixp.tile([BS, HC, WF], f32, tag="r1")
                i1 = mixp.tile([BS, HC, WF], f32, tag="i1")
                nc.scalar.activation(
                    r1, p1r, AF.Relu, bias=addr_v[:, n : n + 1], scale=shp1[:, n : n + 1]
                )
                nc.scalar.activation(
                    i1, p1i, AF.Relu, bias=addi_v[:, n : n + 1], scale=shp1[:, n : n + 1]
                )
                if "2" not in _stages:
                    continue
                # layer 2 (i2 uses biased pre-shrink r2)
                p2r = psum.tile([BS, HC, WF], f32, tag="ps_m")
                nc.tensor.matmul(p2r, lhsT=w2r_sb[:, n, :], rhs=r1, start=True, stop=False)
                nc.tensor.matmul(p2r, lhsT=nw2i_sb[:, n, :], rhs=i1, start=False, stop=True)
                r2b = mixp.tile([BS, HC, WF], f32, tag="r2b")
                nc.scalar.activation(r2b, p2r, AF.Identity, bias=b2r_v[:, n : n + 1])
                p2i = psum.tile([BS, HC, WF], f32, tag="ps_m")
                nc.tensor.matmul(p2i, lhsT=w2i_sb[:, n, :], rhs=r2b, start=True, stop=False)
                nc.tensor.matmul(p2i, lhsT=w2r_sb[:, n, :], rhs=i1, start=False, stop=True)
                # softshrink(r2b): v - clip(v, -lam, lam)  (2 DVE ops, SBUF 2x mode)
                R2 = mixp.tile([BS, HC, WF], f32, tag="R2")
                I2 = mixp.tile([BS, HC, WF], f32, tag="I2")
                sa = mixp.tile([BS, HC, WF], f32, tag="shr_a")
                nc.vector.tensor_scalar(
                    sa, r2b, -LAM, LAM, mybir.AluOpType.max, mybir.AluOpType.min
                )
                nc.vector.tensor_sub(R2, r2b, sa)
                # softshrink(p2i + b2i): relu(v-lam) - relu(-v-lam) straight from PSUM
                sc_ = mixp.tile([BS, HC, WF], f32, tag="shr_a")
                sd_ = mixp.tile([BS, HC, WF], f32, tag="shr_b")
                nc.scalar.activation(
                    sc_, p2i, AF.Relu, bias=b2iml_v[:, n : n + 1]
                )
                nc.scalar.activation(
                    sd_, p2i, AF.Relu, bias=nb2iml_v[:, n : n + 1], scale=-1.0
                )
                nc.vector.tensor_sub(I2, sc_, sd_)
                if "T" not in _stages:
                    continue
                # T: pivot [c, wf] -> [wf, c]; 4 transposes share one psum tile/evict
                pTr = psum.tile([WF, HC, 128], f32, tag="ps_t")
                pTi = psum.tile([WF, HC, 128], f32, tag="ps_t")
                for j in range(HC):
                    nc.tensor.transpose(pTr[:, j, 0:BS], R2[:, j, :], cI_sb[0:BS, 0:BS])
                    nc.tensor.transpose(pTi[:, j, 0:BS], I2[:, j, :], cI_sb[0:BS, 0:BS])
                nc.vector.tensor_copy(Wr_pl[:, h0 : h0 + HC, :], pTr[:, :, 0:BS])
                nc.scalar.copy(Wi_pl[:, h0 : h0 + HC, :], pTi[:, :, 0:BS])

            # ---- stage E': [Pr|Pi] = Z @ [Sr|Si] per channel -> Pbuf bf16 ----
            Pbuf = blockp.tile([H, BS, 2 * H], bf, tag="zpbuf")
            for cp in range(BS // 2 if "E" in _stages else 0):
                c = 2 * cp
                pE = psum.tile([128, 2, 2 * H], f32, tag="ps_a")
                for q in range(2):
                    nc.tensor.matmul(
                        pE[:, q, :], lhsT=Wr_pl[:, :, c + q], rhs=cE1_sb, start=True, stop=False
                    )
                    nc.tensor.matmul(
                        pE[:, q, :], lhsT=Wi_pl[:, :, c + q], rhs=cE2_sb, start=False, stop=True
                    )
                if cp % 2 == 0:
                    nc.vector.tensor_copy(Pbuf[:, c : c + 2, :], pE)
                else:
                    nc.scalar.copy(Pbuf[:, c : c + 2, :], pE)

            # ---- stage D': out = FHr@Pr - FHi@Pi + x ----
            for g in range(BS // 4 if "D" in _stages else 0):
                cg0 = g * 4
                pD = psum.tile([H, 4, W], f32, tag="ps_a")
                nc.tensor.matmul(
                    pD, lhsT=cDr_sb, rhs=Pbuf[:, cg0 : cg0 + 4, 0:H], start=True, stop=False
                )
                nc.tensor.matmul(
                    pD, lhsT=cDi_sb, rhs=Pbuf[:, cg0 : cg0 + 4, H : 2 * H], start=False, stop=True
                )
                ot = outp.tile([H, 4, W], f32, tag="ot")
                nc.vector.tensor_add(ot, pD, X_blk[:, cg0 : cg0 + 4, :])
                for j in range(4):
                    nc.sync.dma_start(outs[c0 + cg0 + j], ot[:, j, :])

        if rep_cm is not None:
            rep_cm.__exit__(None, None, None)

    nc.compile()
    return nc


_CACHE = {}


def _get_program():
    if "nc" not in _CACHE:
        _CACHE["nc"] = _build_program()
    return _CACHE["nc"]


def kernel(**inputs):
    x = np.asarray(inputs["x"], dtype=np.float32)
    t = np.asarray(inputs["t"], dtype=np.float32)
    w1 = np.asarray(inputs["w1"], dtype=np.float32)
    b1 = np.asarray(inputs["b1"], dtype=np.float32)
    w2 = np.asarray(inputs["w2"], dtype=np.float32)
    b2 = np.asarray(inputs["b2"], dtype=np.float32)
    mod_w = np.asarray(inputs["mod_w"], dtype=np.float32)
    mod_b = np.asarray(inputs["mod_b"], dtype=np.float32)

    from concourse.bass_utils import run_bass_kernel_spmd

    nc = _get_program()
    consts = _host_consts()

    in_maps = []
    for core in range(N_CORES):
        b = core // 2
        n0 = (core % 2) * NBL
        cs = slice(n0 * BS, n0 * BS + C)
        rs = slice(n0 * 2 * BS, (n0 + NBL) * 2 * BS)
        im = {
            "xs": np.ascontiguousarray(x[b, cs]),
            "tb": np.ascontiguousarray(t[b]),
            "w1s": np.ascontiguousarray(w1[:, n0 : n0 + NBL]),
            "b1s": np.ascontiguousarray(b1[:, n0 : n0 + NBL]),
            "w2s": np.ascontiguousarray(w2[:, n0 : n0 + NBL]),
            "b2s": np.ascontiguousarray(b2[:, n0 : n0 + NBL]),
            "mwT": np.ascontiguousarray(mod_w[rs].T),
            "mbs": np.ascontiguousarray(mod_b[rs]),
        }
        im.update(consts)
        in_maps.append(im)

    import os as _os
    trace = bool(int(_os.environ.get("AFNO_TRACE", "0")))
    res = run_bass_kernel_spmd(
        nc, in_maps, core_ids=list(range(N_CORES)), trace=trace
    )
    global LAST_RESULTS
    LAST_RESULTS = res

    out = np.empty((B_FULL, DIM, H, W), dtype=np.float32)
    for core in range(N_CORES):
        b = core // 2
        n0 = (core % 2) * NBL
        cs = slice(n0 * BS, n0 * BS + C)
        out[b, cs] = res.results[core]["outs"]
    return out



# revision 5
# speedup vs baseline: 1.6981x; 1.6981x over previous
"""ModAFNO2D layer as a Bass/Tile kernel for 8 Trainium2 NeuronCores.

Sharding: 8 cores = (batch b in 0..3) x (block-half in 0..1). Each core owns one
batch sample and 4 of the 8 FNO blocks (= 384 of 768 channels). The FFT axes are
per-channel and channel blocks never mix, so cores are fully independent - no
collectives; host slices inputs and concatenates outputs.

All matmuls bf16 (fp32 matmuls run as 2 HW passes). Host folds the adaLN
modulation into the layer-1 weights (column scaling + bias) and rewrites layer 2
with composite weights so both its outputs are direct functions of (r1, i1);
layer 2 then runs "data-stationary" (spectrum slice as the PE stationary), which
lands its output already transposed to [wf, c] - eliminating all explicit PE
transposes. Stage A exploits the Hermitian symmetry of the H-axis FFT of the
real input (only h' 0..64 computed; stage B mirrors with conjugate weights).

Per-core pipeline per block (96 channels):
  A : Zbuf[w, c, (h'r|h'i)] = X_c^T @ [Fr|Fi][:, 0:65]     (FFT along H)
  B : arch[c, hc, (wfr|wfi)] per h'                          (rFFT along W)
  l1: weight-stationary block matmul, modulation pre-folded, relu
  l2: data-stationary -> psum [wf, hc, (r|i)]; softshrink on evict
  E': Pbuf[h', c, (Pr|Pi)] = W_c @ [Sr|Si]                   (inverse rFFT W)
  D': out[h, c, w] = FHr@Pr + FHi'@Pi + x                    (inverse FFT H)
"""

import numpy as np
import ml_dtypes

BF16 = ml_dtypes.bfloat16

DIM = 768
NB = 8
BS = 96
LAM = 0.01
B_FULL = 4
H = 128
W = 128
WF = W // 2 + 1  # 65
HF = H // 2 + 1  # 65 (Hermitian-reduced H freqs)
NBL = 4          # blocks per core
C = NBL * BS     # 384 channels per core
N_CORES = 8
HC = 4           # h' rows per fused B/mix chunk


def _host_consts():
    jh = np.arange(H)
    F = np.exp(-2j * np.pi * np.outer(jh, jh) / H)
    R = np.exp(-2j * np.pi * np.outer(np.arange(WF), np.arange(W)) / W) / 128.0
    RrT, RiT = np.ascontiguousarray(R.real.T), np.ascontiguousarray(R.imag.T)
    FH = np.conj(F)
    cw = np.ones(WF)
    cw[1:-1] = 2.0
    S = (cw[:, None] * np.exp(2j * np.pi * np.outer(np.arange(WF), np.arange(W)) / W)) / 128.0
    cB2 = np.concatenate([-RiT, RrT], 1)
    consts = {
        "cFh": np.concatenate([F.real[:, :HF], F.imag[:, :HF]], 1).astype(BF16),  # [128, 130]
        "cB1": np.concatenate([RrT, RiT], 1).astype(BF16),                 # [128, 130]
        "cB2": cB2.astype(BF16),                                           # [128, 130]
        "cB2n": (-cB2).astype(BF16),                                       # [128, 130]
        "cE1": np.concatenate([S.real, S.imag], 1).astype(BF16),           # [65, 256]
        "cE2": np.concatenate([-S.imag, S.real], 1).astype(BF16),          # [65, 256]
        "cDr": FH.real.astype(BF16),                                       # [128, 128]
        "cDi": (-FH.imag).astype(BF16),                                    # [128, 128]
    }
    return consts


def _build_program():
    from contextlib import ExitStack

    import concourse.bass as bass  # noqa: F401
    import concourse.mybir as mybir
    import concourse.tile as tile
    from concourse import bacc

    f32 = mybir.dt.float32
    bf = mybir.dt.bfloat16
    AF = mybir.ActivationFunctionType
    ALU = mybir.AluOpType

    nc = bacc.Bacc("TRN2", target_bir_lowering=False, debug=False)

    xh = nc.dram_tensor("xh", [H, C, W], bf, kind="ExternalInput")
    w1k = nc.dram_tensor("w1k", [4, NBL, BS + 1, BS], bf, kind="ExternalInput")
    w2a = nc.dram_tensor("w2a", [NBL, BS + 1, 2 * BS], bf, kind="ExternalInput")
    w2b = nc.dram_tensor("w2b", [NBL, BS, 2 * BS], bf, kind="ExternalInput")
    cFh = nc.dram_tensor("cFh", [H, 2 * HF], bf, kind="ExternalInput")
    cB1 = nc.dram_tensor("cB1", [W, 2 * WF], bf, kind="ExternalInput")
    cB2 = nc.dram_tensor("cB2", [W, 2 * WF], bf, kind="ExternalInput")
    cB2n = nc.dram_tensor("cB2n", [W, 2 * WF], bf, kind="ExternalInput")
    cE1 = nc.dram_tensor("cE1", [WF, 2 * W], bf, kind="ExternalInput")
    cE2 = nc.dram_tensor("cE2", [WF, 2 * W], bf, kind="ExternalInput")
    cDr = nc.dram_tensor("cDr", [H, H], bf, kind="ExternalInput")
    cDi = nc.dram_tensor("cDi", [H, H], bf, kind="ExternalInput")
    outs = nc.dram_tensor("outs", [H, C, W], f32, kind="ExternalOutput")

    with ExitStack() as ctx:
        tc = ctx.enter_context(tile.TileContext(nc))
        consts = ctx.enter_context(tc.tile_pool(name="consts", bufs=1))
        xpool = ctx.enter_context(tc.tile_pool(name="xpool", bufs=2))
        zpool = ctx.enter_context(tc.tile_pool(name="zpool", bufs=1))
        wpool = ctx.enter_context(tc.tile_pool(name="wpool", bufs=1))
        ppool = ctx.enter_context(tc.tile_pool(name="ppool", bufs=1))
        mixp = ctx.enter_context(tc.tile_pool(name="mixp", bufs=2))
        outp = ctx.enter_context(tc.tile_pool(name="outp", bufs=3))
        psum = ctx.enter_context(tc.tile_pool(name="psum", bufs=2, space="PSUM"))

        # ---- constants into SBUF ----
        cFh_sb = consts.tile([H, 2 * HF], bf)
        nc.sync.dma_start(cFh_sb, cFh[:])
        cB1_sb = consts.tile([W, 2 * WF], bf)
        nc.sync.dma_start(cB1_sb, cB1[:])
        cB2_sb = consts.tile([W, 2 * WF], bf)
        nc.sync.dma_start(cB2_sb, cB2[:])
        cB2n_sb = consts.tile([W, 2 * WF], bf)
        nc.sync.dma_start(cB2n_sb, cB2n[:])
        cE1_sb = consts.tile([WF, 2 * W], bf)
        nc.sync.dma_start(cE1_sb, cE1[:])
        cE2_sb = consts.tile([WF, 2 * W], bf)
        nc.sync.dma_start(cE2_sb, cE2[:])
        cDr_sb = consts.tile([H, H], bf)
        nc.sync.dma_start(cDr_sb, cDr[:])
        cDi_sb = consts.tile([H, H], bf)
        nc.sync.dma_start(cDi_sb, cDi[:])

        # layer-1 weights (modulation folded): [97, kind, n, 96]
        # kinds: 0=w1r*sh (+addr bias row), 1=-w1i*sh (0 row),
        #        2=w1i*sh (+addi bias row), 3=w1r*sh (0 row)
        w1_sb = consts.tile([BS + 1, 4, NBL, BS], bf)
        nc.sync.dma_start(w1_sb, w1k[:].rearrange("k n d c -> d k n c"))
        # layer-2 rhs: A=[w2r|W2ri] + bias row, B=[-w2i|W2c]
        w2a_sb = consts.tile([BS + 1, NBL, 2 * BS], bf)
        nc.sync.dma_start(w2a_sb, w2a[:].rearrange("n d c -> d n c"))
        w2b_sb = consts.tile([BS, NBL, 2 * BS], bf)
        nc.sync.dma_start(w2b_sb, w2b[:].rearrange("n d c -> d n c"))
        lamn = consts.tile([128, 1], f32)
        nc.vector.memset(lamn, -LAM)

        # ---- main per-block pipeline ----
        for n in range(NBL):
            c0 = n * BS

            # resident X for this block: [h, c, w] bf16
            X_blk = xpool.tile([H, BS, W], bf, tag="xblk")
            nc.sync.dma_start(X_blk, xh[:, c0 : c0 + BS, :])

            # ---- stage A: Z^T = X_c^T @ [Fr|Fi] (h' 0..64) ----
            Zbuf = zpool.tile([W, BS, 2 * HF], bf, tag="zbuf")
            for cp in range(BS // 2):
                c = 2 * cp
                pA = psum.tile([128, 2, 2 * HF], f32, tag="ps_x")
                nc.tensor.matmul(pA[:, 0, :], lhsT=X_blk[:, c, :], rhs=cFh_sb, start=True, stop=True)
                nc.tensor.matmul(pA[:, 1, :], lhsT=X_blk[:, c + 1, :], rhs=cFh_sb, start=True, stop=True)
                if cp % 2 == 0:
                    nc.vector.tensor_copy(Zbuf[:, c : c + 2, :], pA)
                else:
                    nc.scalar.copy(Zbuf[:, c : c + 2, :], pA)

            # ---- fused B -> l1 -> l2(+shrink) per chunk of HC h' rows ----
            Wr_pl = wpool.tile([WF, H, BS], bf, tag="wrpl")
            Wi_pl = wpool.tile([WF, H, BS], bf, tag="wipl")
            for ch_i in range(H // HC):
                h0 = ch_i * HC
                arch = mixp.tile([BS + 1, HC, 2 * WF], bf, tag="arch")
                nc.gpsimd.memset(arch[BS : BS + 1, :, :], 1.0)
                for j2 in range(HC // 2):
                    pB = psum.tile([BS, 2, 2 * WF], f32, tag="ps_x")
                    for j in range(2):
                        hj = h0 + j2 * 2 + j
                        m = hj if hj <= 64 else 128 - hj
                        rhs2 = cB2_sb if hj <= 64 else cB2n_sb
                        nc.tensor.matmul(
                            pB[:, j, :], lhsT=Zbuf[:, :, m], rhs=cB1_sb,
                            start=True, stop=False,
                        )
                        nc.tensor.matmul(
                            pB[:, j, :], lhsT=Zbuf[:, :, HF + m], rhs=rhs2,
                            start=False, stop=True,
                        )
                    if j2 % 2 == 0:
                        nc.vector.tensor_copy(arch[0:BS, j2 * 2 : j2 * 2 + 2, :], pB)
                    else:
                        nc.scalar.copy(arch[0:BS, j2 * 2 : j2 * 2 + 2, :], pB)
                Ar = arch[:, :, 0:WF]
                Ai = arch[:, :, WF : 2 * WF]
                # layer 1 (bias via ones-row of arch x bias-row of weights)
                p1r = psum.tile([BS, HC, WF], f32, tag="ps_1")
                nc.tensor.matmul(p1r, lhsT=w1_sb[:, 0, n, :], rhs=Ar, start=True, stop=False)
                nc.tensor.matmul(p1r, lhsT=w1_sb[:, 1, n, :], rhs=Ai, start=False, stop=True)
                p1i = psum.tile([BS, HC, WF], f32, tag="ps_1")
                nc.tensor.matmul(p1i, lhsT=w1_sb[:, 2, n, :], rhs=Ar, start=True, stop=False)
                nc.tensor.matmul(p1i, lhsT=w1_sb[:, 3, n, :], rhs=Ai, start=False, stop=True)
                r1a = mixp.tile([BS + 1, HC, WF], bf, tag="r1a")
                i1 = mixp.tile([BS, HC, WF], bf, tag="i1")
                nc.scalar.activation(r1a[0:BS], p1r, AF.Relu)
                nc.gpsimd.memset(r1a[BS : BS + 1, :, :], 1.0)
                nc.vector.tensor_scalar_max(i1, p1i, 0.0)
                # layer 2, data-stationary: out[wf, (r2|i2)] per h'
                p2 = psum.tile([WF, HC, 256], f32, tag="ps_2")
                for j in range(HC):
                    nc.tensor.matmul(
                        p2[:, j, 0 : 2 * BS], lhsT=r1a[:, j, :], rhs=w2a_sb[:, n, :],
                        start=True, stop=False,
                    )
                    nc.tensor.matmul(
                        p2[:, j, 0 : 2 * BS], lhsT=i1[:, j, :], rhs=w2b_sb[:, n, :],
                        start=False, stop=True,
                    )
                # softshrink(v) = relu(v - lam) + min(v + lam, 0)
                tur = mixp.tile([WF, HC, BS], bf, tag="tur")
                tmr = mixp.tile([WF, HC, BS], bf, tag="tmr")
                tui = mixp.tile([WF, HC, BS], bf, tag="tui")
                tmi = mixp.tile([WF, HC, BS], bf, tag="tmi")
                nc.scalar.activation(tur, p2[:, :, 0:BS], AF.Relu, bias=lamn[0:WF])
                nc.vector.tensor_scalar(tmr, p2[:, :, 0:BS], LAM, 0.0, ALU.add, ALU.min)
                nc.scalar.activation(tui, p2[:, :, BS : 2 * BS], AF.Relu, bias=lamn[0:WF])
                nc.vector.tensor_scalar(tmi, p2[:, :, BS : 2 * BS], LAM, 0.0, ALU.add, ALU.min)
                nc.vector.tensor_add(Wr_pl[:, h0 : h0 + HC, :], tur, tmr)
                nc.gpsimd.tensor_add(Wi_pl[:, h0 : h0 + HC, :], tui, tmi)

            # ---- stage E': [Pr|Pi] = W_c @ [Sr|Si] per channel ----
            Pbuf = ppool.tile([H, BS, 2 * H], bf, tag="pbuf")
            for cp in range(BS // 2):
                c = 2 * cp
                pE = psum.tile([128, 2, 2 * H], f32, tag="ps_x")
                for q in range(2):
                    nc.tensor.matmul(
                        pE[:, q, :], lhsT=Wr_pl[:, :, c + q], rhs=cE1_sb, start=True, stop=False
                    )
                    nc.tensor.matmul(
                        pE[:, q, :], lhsT=Wi_pl[:, :, c + q], rhs=cE2_sb, start=False, stop=True
                    )
                if cp % 2 == 0:
                    nc.vector.tensor_copy(Pbuf[:, c : c + 2, :], pE)
                else:
                    nc.scalar.copy(Pbuf[:, c : c + 2, :], pE)

            # ---- stage D': out = FHr@Pr - FHi@Pi + x ----
            for g in range(BS // 4):
                cg0 = g * 4
                pD = psum.tile([H, 4, W], f32, tag="ps_x")
                nc.tensor.matmul(
                    pD, lhsT=cDr_sb, rhs=Pbuf[:, cg0 : cg0 + 4, 0:H], start=True, stop=False
                )
                nc.tensor.matmul(
                    pD, lhsT=cDi_sb, rhs=Pbuf[:, cg0 : cg0 + 4, H : 2 * H], start=False, stop=True
                )
                ot = outp.tile([H, 4, W], f32, tag="ot")
                nc.vector.tensor_add(ot, pD, X_blk[:, cg0 : cg0 + 4, :])
                nc.sync.dma_start(outs[:, c0 + cg0 : c0 + cg0 + 4, :], ot)

    nc.compile()
    return nc


_CACHE = {}


def _get_program():
    if "nc" not in _CACHE:
        _CACHE["nc"] = _build_program()
    return _CACHE["nc"]


def kernel(**inputs):
    x = np.asarray(inputs["x"], dtype=np.float32)
    t = np.asarray(inputs["t"], dtype=np.float32)
    w1 = np.asarray(inputs["w1"], dtype=np.float32)
    b1 = np.asarray(inputs["b1"], dtype=np.float32)
    w2 = np.asarray(inputs["w2"], dtype=np.float32)
    b2 = np.asarray(inputs["b2"], dtype=np.float32)
    mod_w = np.asarray(inputs["mod_w"], dtype=np.float32)
    mod_b = np.asarray(inputs["mod_b"], dtype=np.float32)

    from concourse.bass_utils import run_bass_kernel_spmd

    nc = _get_program()
    consts = _host_consts()

    # adaLN modulation on host: mod = silu(t) @ mod_w.T + mod_b
    st = t / (1.0 + np.exp(-t))
    mod = st @ mod_w.T + mod_b                      # (B, 2*DIM)
    mod = mod.reshape(B_FULL, NB, 2 * BS)
    shift, scale = mod[..., :BS], mod[..., BS:]     # (B, NB, BS)

    in_maps = []
    for core in range(N_CORES):
        b = core // 2
        n0 = (core % 2) * NBL
        cs = slice(n0 * BS, n0 * BS + C)

        # layer-1 weights with modulation folded (column scale + bias)
        sh = shift[b, n0 : n0 + NBL] + 1.0          # (NBL, BS) per c_out
        sc = scale[b, n0 : n0 + NBL]
        w1r = w1[0, n0 : n0 + NBL] * sh[:, None, :]  # (NBL, BS, BS)
        w1i = w1[1, n0 : n0 + NBL] * sh[:, None, :]
        addr = b1[0, n0 : n0 + NBL] * sh + sc        # (NBL, BS)
        addi = b1[1, n0 : n0 + NBL] * sh + sc
        zr = np.zeros((NBL, 1, BS), np.float32)
        w1k = np.stack([
            np.concatenate([w1r, addr[:, None, :]], 1),
            np.concatenate([-w1i, zr], 1),
            np.concatenate([w1i, addi[:, None, :]], 1),
            np.concatenate([w1r, zr], 1),
        ])                                           # (4, NBL, 97, 96)

        # layer-2 composites: r2 = r1@w2r - i1@w2i + b2r
        # i2 = r1@(w2r@w2i) + i1@(w2r - w2i@w2i) + (b2r@w2i + b2i)
        w2r = w2[0, n0 : n0 + NBL]
        w2i = w2[1, n0 : n0 + NBL]
        b2r = b2[0, n0 : n0 + NBL]
        b2i = b2[1, n0 : n0 + NBL]
        W2ri = np.einsum("ndk,nkm->ndm", w2r, w2i)
        W2c = w2r - np.einsum("ndk,nkm->ndm", w2i, w2i)
        b2ip = np.einsum("nk,nkm->nm", b2r, w2i) + b2i
        w2a = np.concatenate([
            np.concatenate([w2r, W2ri], 2),
            np.concatenate([b2r[:, None, :], b2ip[:, None, :]], 2),
        ], 1)                                        # (NBL, 97, 192)
        w2b = np.concatenate([-w2i, W2c], 2)         # (NBL, 96, 192)

        im = {
            "xh": np.ascontiguousarray(
                x[b, cs].transpose(1, 0, 2)).astype(BF16),   # [H, C, W]
            "w1k": w1k.astype(BF16),
            "w2a": w2a.astype(BF16),
            "w2b": w2b.astype(BF16),
        }
        im.update(consts)
        in_maps.append(im)

    import os as _os
    trace = bool(int(_os.environ.get("AFNO_TRACE", "0")))
    res = run_bass_kernel_spmd(
        nc, in_maps, core_ids=list(range(N_CORES)), trace=trace
    )
    global LAST_RESULTS
    LAST_RESULTS = res

    out = np.empty((B_FULL, DIM, H, W), dtype=np.float32)
    for core in range(N_CORES):
        b = core // 2
        n0 = (core % 2) * NBL
        cs = slice(n0 * BS, n0 * BS + C)
        out[b, cs] = res.results[core]["outs"].transpose(1, 0, 2)
    return out
